# revision 33
# baseline (speedup 1.0000x reference)
"""Null-KV MQA attention (LN'd Q path, causal, per-head bias) on 8 trn2 cores.

Sharding: heads tensor-parallel (2 heads/core x 8 cores), batch replicated.
Each core computes q = LN(x) @ WqT for its 2 heads, the shared k/v (MQA),
its head-pair's scores/softmax/attn@v, and a partial output projection
through its Wo column slice; the host sums the 8 partial outputs.

Device layout notes:
  * Scores are computed transposed: s^T[j, i] tiles [128, 512] so that the
    softmax denominator and attn@v contract j on the partition axis.
  * Null keys/values are permuted to the END of the key axis (positions
    2048, 2049) so token key positions stay 128-aligned; softmax is
    permutation-invariant so this matches the reference exactly.
  * Causal masking skips fully-dead j-tiles (half the attention work) and
    zeroes boundary tiles with gpsimd affine_select after exp.
  * attn_bias is injected into PSUM with an identity matmul (start=True)
    and the scores matmul accumulates on top (start=False), so no
    elementwise engine pass touches the [j, i] bias tensor.
  * LN is folded into the q projection: an extra contraction row carries
    -mu (per token) against the column sums of Wq, and the 1/std scale is
    applied to q^T afterwards (per-column broadcast via gpsimd).
  * The softmax denominator rides along attn@v as a 65th lhsT column of
    ones; normalization divides by it after the accumulation.
"""

import sys

for _p in ("/opt/trn_rl_repo",):
    if _p not in sys.path:
        sys.path.append(_p)

import numpy as np
import ml_dtypes

BF16 = ml_dtypes.bfloat16

B = 2
N = 2048
DIM = 1024
HEADS = 16
DH = 64
NNULL = 2
HL = 2                    # heads per core
NCORES = 8
JT = 17                   # j tiles of 128 (2048 tokens + 2 nulls + pad)
JPAD = JT * 128
CC = 4                    # 512-wide query chunks per batch
SCALE = DH ** -0.5
EPS = 1e-5

# How attn_bias reaches PSUM: "mm" = identity-matmul injection (sets
# has_written, guaranteed accumulate semantics); "dve" = DVE copy into PSUM
# before the scores matmul (cheaper on PE iff walrus handles has_written).
BIAS_VIA = "mm"

_CACHE = {}


def _build_nc():
    import concourse.bass as bass
    import concourse.tile as tile
    from concourse import bacc, mybir
    from concourse.masks import make_identity

    fp32 = mybir.dt.float32
    bf16 = mybir.dt.bfloat16
    Alu = mybir.AluOpType
    Act = mybir.ActivationFunctionType

    nc = bacc.Bacc(None)

    xT = nc.dram_tensor("xT", [DIM, B * N], bf16, kind="ExternalInput")
    wqT = nc.dram_tensor("wqT", [DIM, 128], bf16, kind="ExternalInput")
    wkvT = nc.dram_tensor("wkvT", [DIM, 128], bf16, kind="ExternalInput")
    woT = nc.dram_tensor("woT", [128, DIM], bf16, kind="ExternalInput")
    biasT = nc.dram_tensor("biasT", [HL, JPAD, N], bf16, kind="ExternalInput")
    lnw = nc.dram_tensor("lnw", [128, 8], fp32, kind="ExternalInput")
    lnb = nc.dram_tensor("lnb", [128, 8], fp32, kind="ExternalInput")
    nullkT = nc.dram_tensor("nullkT", [DH, NNULL], bf16, kind="ExternalInput")
    nullv = nc.dram_tensor("nullv", [NNULL, DH], bf16, kind="ExternalInput")
    out = nc.dram_tensor("out", [B * N, DIM], fp32, kind="ExternalOutput")

    with tile.TileContext(nc) as tc:
        with (
            tc.tile_pool(name="const", bufs=1) as const,
            tc.tile_pool(name="rows", bufs=4) as rows,
        ):
            ident = const.tile([128, 128], bf16)
            make_identity(nc, ident)
            ident32 = const.tile([128, 128], fp32)
            make_identity(nc, ident32)
            ones_col = const.tile([128, 1], bf16)
            nc.gpsimd.memset(ones_col, 1.0)
            eps_sb = const.tile([1, 1], fp32)
            nc.gpsimd.memset(eps_sb, EPS)
            lnscale_sb = const.tile([1, 1], fp32)
            nc.gpsimd.memset(lnscale_sb, float(np.log(SCALE)))

            lnw_sb = const.tile([128, 8], fp32)
            nc.sync.dma_start(out=lnw_sb, in_=lnw[:, :])
            lnb_sb = const.tile([128, 8], fp32)
            nc.sync.dma_start(out=lnb_sb, in_=lnb[:, :])
            lnb_bf = const.tile([128, 8], bf16)
            nc.vector.tensor_copy(out=lnb_bf, in_=lnb_sb)

            wq_t = []
            for kt in range(8):
                w = const.tile([128, 128], bf16, tag=f"wq{kt}")
                nc.sync.dma_start(out=w, in_=wqT[128 * kt:128 * (kt + 1), :])
                # fold ln_w into Wq (per-d scale)
                nc.vector.tensor_scalar_mul(w, w, lnw_sb[:, kt:kt + 1])
                wq_t.append(w)
            wkv_t = []
            for kt in range(8):
                w = const.tile([128, 128], bf16, tag=f"wkv{kt}")
                nc.sync.dma_start(out=w, in_=wkvT[128 * kt:128 * (kt + 1), :])
                wkv_t.append(w)
            wo_sb = const.tile([128, DIM], bf16)
            nc.sync.dma_start(out=wo_sb, in_=woT[:, :])

            # k^T slabs per batch [64 dh, JPAD] (base partition 0 for matmul)
            kT_slab = []
            for b in range(B):
                kt_b = const.tile([64, JPAD], bf16, tag=f"kT{b}")
                nc.gpsimd.memset(kt_b[:, N:JPAD], 0.0)
                nc.sync.dma_start(out=kt_b[:, N:N + NNULL], in_=nullkT[:, :])
                kT_slab.append(kt_b)
            # v slabs per j-tile: [128 j, 2 batches, 64 v + 1 ones]
            v_t = []
            for jt in range(JT):
                v = const.tile([128, B, DH + 1], bf16, tag=f"v{jt}")
                if jt == JT - 1:
                    nc.gpsimd.memset(v, 0.0)
                for b in range(B):
                    nc.gpsimd.memset(v[:, b, DH:DH + 1], 1.0)
                    if jt == JT - 1:
                        nc.sync.dma_start(
                            out=v[0:NNULL, b, 0:DH], in_=nullv[:, :]
                        )
                v_t.append(v)

            # q^T slabs per local head [64 dh, B*N]
            qslab = [
                const.tile([64, B * N], bf16, tag=f"q{h}", name=f"qslab{h}")
                for h in range(HL)
            ]

            # column sums of ln_w-folded Wq (for the -mu contraction row)
            # and Wq' @ ln_b (per-output-bias of q)
            with tc.tile_pool(name="ps0", bufs=1, space="PSUM") as ps0:
                csum_ps = ps0.tile([128, 1], fp32, tag="misc")
                for kt in range(8):
                    nc.tensor.matmul(
                        csum_ps, wq_t[kt], ones_col,
                        start=(kt == 0), stop=(kt == 7),
                    )
                csum_sb = const.tile([128, 1], fp32)
                nc.scalar.copy(csum_sb, csum_ps)
                bq_ps = ps0.tile([128, 1], fp32, tag="misc")
                for kt in range(8):
                    nc.tensor.matmul(
                        bq_ps, wq_t[kt], lnb_bf[:, kt:kt + 1],
                        start=(kt == 0), stop=(kt == 7),
                    )
                bq_sb = const.tile([128, 1], fp32)
                nc.scalar.copy(bq_sb, bq_ps)
                csumT_ps = ps0.tile([1, 128], fp32, tag="misc")
                nc.tensor.transpose(csumT_ps, csum_sb, ident32)
                csum_row = const.tile([1, 128], bf16)
                nc.scalar.copy(csum_row, csumT_ps)

            # ---- Phase 1: LN stats + q/kv projections per 512-token chunk
            with (
                tc.tile_pool(name="xp", bufs=3) as xp,
                tc.tile_pool(name="p1a", bufs=1, space="PSUM") as p1a,
                tc.tile_pool(name="p1b", bufs=2, space="PSUM") as p1b,
            ):
                for c in [b * CC + cc for cc in range(CC) for b in range(B)]:
                    b, cc = divmod(c, CC)
                    col0 = N * b + 512 * cc
                    # one DMA fetches all 8 k-tile blocks of this chunk
                    xc = xp.tile([128, 8, 512], bf16, tag="xc", name="xc")
                    src_ap = bass.AP(
                        tensor=xT[0:1, 0:1].tensor, offset=col0,
                        ap=[[B * N, 128], [128 * B * N, 8], [1, 512]],
                    )
                    nc.sync.dma_start(out=xc, in_=src_ap)
                    x2c = xp.tile([128, 8, 512], bf16, tag="x2c", name="x2c")
                    nc.vector.tensor_tensor(
                        out=x2c[:, :, :], in0=xc[:, :, :], in1=xc[:, :, :],
                        op=Alu.mult,
                    )
                    xt = [xc[:, kt, :] for kt in range(8)]
                    sumx_ps = p1a.tile([1, 512], fp32, tag="sumx")
                    for kt in range(8):
                        nc.tensor.matmul(
                            sumx_ps, ones_col, xc[:, kt, :],
                            start=(kt == 0), stop=(kt == 7),
                        )
                    sumx2_ps = p1a.tile([1, 512], fp32, tag="sumx2")
                    for kt in range(8):
                        nc.tensor.matmul(
                            sumx2_ps, ones_col, x2c[:, kt, :],
                            start=(kt == 0), stop=(kt == 7),
                        )
                    negmu = rows.tile([1, 512], bf16, tag="negmu")
                    nc.vector.tensor_scalar_mul(negmu, sumx_ps, -1.0 / DIM)
                    mu2 = rows.tile([1, 512], fp32, tag="mu2")
                    nc.vector.tensor_tensor(
                        out=mu2, in0=negmu, in1=negmu, op=Alu.mult
                    )
                    var = rows.tile([1, 512], fp32, tag="var")
                    nc.vector.scalar_tensor_tensor(
                        out=var, in0=sumx2_ps, scalar=1.0 / DIM, in1=mu2,
                        op0=Alu.mult, op1=Alu.subtract,
                    )
                    lv = rows.tile([1, 512], fp32, tag="lv")
                    nc.scalar.activation(lv, var, Act.Ln, bias=eps_sb)
                    # r = SCALE / sqrt(var+eps) = exp(-0.5*ln(var+eps) + ln(SCALE))
                    rec = rows.tile([1, 512], fp32, tag="rec")
                    nc.scalar.activation(rec, lv, Act.Exp, scale=-0.5, bias=lnscale_sb)
                    rb = rows.tile([128, 512], fp32, tag="rb")
                    nc.gpsimd.partition_broadcast(rb, rec, channels=128)

                    qu_ps = p1b.tile([128, 512], fp32, tag="qu")
                    for kt in range(8):
                        nc.tensor.matmul(
                            qu_ps, wq_t[kt], xt[kt], start=(kt == 0), stop=False
                        )
                    nc.tensor.matmul(qu_ps, csum_row, negmu, start=False, stop=True)
                    for h in range(HL):
                        qsl = qslab[h][:, col0:col0 + 512]
                        hs = slice(64 * h, 64 * h + 64)
                        nc.vector.tensor_tensor(
                            out=qsl, in0=qu_ps[hs, :], in1=rb[hs, :], op=Alu.mult
                        )
                        nc.vector.tensor_scalar_add(qsl, qsl, bq_sb[hs, :])

                    kvu_ps = p1b.tile([128, 512], fp32, tag="kvu")
                    for kt in range(8):
                        nc.tensor.matmul(
                            kvu_ps, wkv_t[kt], xt[kt],
                            start=(kt == 0), stop=(kt == 7),
                        )
                    tok0 = 512 * cc
                    nc.scalar.copy(kT_slab[b][:, tok0:tok0 + 512], kvu_ps[0:64, :])
                    vt_sb = xp.tile([64, 512], bf16, tag="vtsb", name="vt_sb")
                    nc.vector.tensor_copy(out=vt_sb, in_=kvu_ps[64:128, :])
                    for s in range(4):
                        vt_ps = p1b.tile([128, DH], bf16, tag="vt")
                        nc.tensor.transpose(
                            vt_ps, vt_sb[:, 128 * s:128 * (s + 1)],
                            ident[0:64, 0:64],
                        )
                        jt = 4 * cc + s
                        nc.vector.tensor_copy(out=v_t[jt][:, b, 0:DH], in_=vt_ps)

            # ---- Phase 2: scores, softmax, attn@v, Wo partial projection
            # Loop order cc -> jt -> (h, b) keeps consecutive PE matmuls on
            # the same lhsT (ident for bias injects, kT per batch) so weight
            # loads amortize and the PE pipeline stays dense.
            with (
                tc.tile_pool(name="sl", bufs=1) as sl,
                tc.tile_pool(name="bp", bufs=8) as bp,
                tc.tile_pool(name="wop", bufs=2) as wop,
                tc.tile_pool(name="aop", bufs=3) as aop,
                tc.tile_pool(name="p2s", bufs=1, space="PSUM") as p2s,
                tc.tile_pool(name="p2av", bufs=2, space="PSUM") as p2av,
            ):
                pending_wo = [None]

                def emit_wo(ao, cc):
                    for b in range(B):
                        for tt in range(4):
                            for w in range(2):
                                wo_ps = p2av.tile(
                                    [128, 512], fp32, tag=f"av{w}", name="wo_ps"
                                )
                                nc.tensor.matmul(
                                    wo_ps, ao[b][:, 128 * tt:128 * (tt + 1)],
                                    wo_sb[:, 512 * w:512 * (w + 1)],
                                    start=True, stop=True,
                                )
                                wo_out = wop.tile(
                                    [128, 512], fp32, tag=f"wout{w}", name="wo_out"
                                )
                                nc.vector.tensor_copy(out=wo_out, in_=wo_ps)
                                r0 = N * b + 512 * cc + 128 * tt
                                nc.sync.dma_start(
                                    out=out[r0:r0 + 128, 512 * w:512 * (w + 1)],
                                    in_=wo_out,
                                )

                for cc in range(CC):
                    live = list(range(4 * cc + 4)) + [JT - 1]
                    nlive = len(live)
                    ao = {}
                    for b in range(B):
                        ao[b] = aop.tile([128, 512], bf16, tag=f"ao{b}", name=f"ao{b}")
                    et_tiles = {h: {} for h in range(HL)}
                    for idx, jt in enumerate(live):
                        boundary = jt >= 4 * cc and jt != JT - 1
                        # live column range within this 512-chunk
                        c0 = 128 * (jt - 4 * cc) if boundary else 0
                        bt = bp.tile([128, HL, 512], bf16, tag="bias", name="bt")
                        nc.sync.dma_start(
                            out=bt,
                            in_=biasT[:, 128 * jt:128 * (jt + 1),
                                      512 * cc:512 * (cc + 1)].rearrange(
                                          "h j i -> j h i"),
                        )
                        s_ps = {}
                        for h in range(HL):
                            sp = p2s.tile(
                                [128, B, 512], fp32, tag=f"s{h}", name="sp"
                            )
                            s_ps[h] = sp
                        if jt != JT - 1:
                            # bias injection: 4 matmuls sharing the ident lhsT
                            for h in range(HL):
                                for b in range(B):
                                    nc.tensor.matmul(
                                        s_ps[h][:, b, c0:512], ident,
                                        bt[:, h, c0:512],
                                        start=True, stop=False,
                                    )
                        for b in range(B):
                            # scores: 2 matmuls sharing the kT lhsT
                            for h in range(HL):
                                nc.tensor.matmul(
                                    s_ps[h][:, b, c0:512],
                                    kT_slab[b][:, 128 * jt:128 * (jt + 1)],
                                    qslab[h][:, N * b + 512 * cc + c0:
                                             N * b + 512 * (cc + 1)],
                                    start=(jt == JT - 1), stop=True,
                                )
                        for h in range(HL):
                            et = sl.tile(
                                [128, B, 512], bf16, tag=f"e{h}_{jt}", name="et"
                            )
                            nc.scalar.activation(
                                et[:, :, c0:512], s_ps[h][:, :, c0:512], Act.Exp
                            )
                            if c0 > 0:
                                nc.gpsimd.memset(et[:, :, 0:c0], 0.0)
                            if boundary:
                                nc.gpsimd.affine_select(
                                    out=et[:, :, c0:512], in_=et[:, :, c0:512],
                                    compare_op=Alu.is_ge, fill=0.0,
                                    base=512 * cc + c0 - 128 * jt,
                                    channel_multiplier=-1,
                                    pattern=[[0, B], [1, 512 - c0]],
                                )
                            elif jt == JT - 1:
                                nc.gpsimd.affine_select(
                                    out=et, in_=et,
                                    compare_op=Alu.is_ge, fill=0.0,
                                    base=NNULL - 1, channel_multiplier=-1,
                                    pattern=[[0, B], [0, 512]],
                                )
                            et_tiles[h][jt] = et
                    # attn@v batch-major: batch 0 chains finish first so its
                    # normalize (recip/broadcast/mult) overlaps batch 1 on PE,
                    # and Wo(b0) issues without waiting
                    for b in range(B):
                        av_ps = {}
                        for h in range(HL):
                            avp = p2av.tile(
                                [DH + 1, 512], fp32, tag=f"av{b}", name="avp"
                            )
                            av_ps[h] = avp
                        for i, jt in enumerate(live):
                            for h in range(HL):
                                nc.tensor.matmul(
                                    av_ps[h], v_t[jt][:, b, :],
                                    et_tiles[h][jt][:, b, :],
                                    start=(i == 0), stop=(i == nlive - 1),
                                )
                        for h in range(HL):
                            rd = rows.tile([1, 512], fp32, tag="rd")
                            nc.vector.reciprocal(rd, av_ps[h][DH:DH + 1, :])
                            db = rows.tile([64, 512], fp32, tag="db")
                            nc.gpsimd.partition_broadcast(db, rd, channels=64)
                            nc.vector.tensor_tensor(
                                out=ao[b][64 * h:64 * h + 64, :],
                                in0=av_ps[h][0:DH, :], in1=db, op=Alu.mult,
                            )
                    emit_wo(ao, cc)

    nc.compile()
    return nc


def _get_nc():
    if "nc" not in _CACHE:
        _CACHE["nc"] = _build_nc()
    return _CACHE["nc"]


def _prep_in_maps(inputs):
    x = np.asarray(inputs["x"], dtype=np.float32)
    Wq = np.asarray(inputs["Wq"], dtype=np.float32)
    Wkv = np.asarray(inputs["Wkv"], dtype=np.float32)
    Wo = np.asarray(inputs["Wo"], dtype=np.float32)
    attn_bias = np.asarray(inputs["attn_bias"], dtype=np.float32)
    null_kv = np.asarray(inputs["null_kv"], dtype=np.float32)
    ln_w = np.asarray(inputs["ln_w"], dtype=np.float32)
    ln_b = np.asarray(inputs["ln_b"], dtype=np.float32)

    xT = np.ascontiguousarray(x.reshape(B * N, DIM).T).astype(BF16)
    wkvT = np.ascontiguousarray(Wkv.T).astype(BF16)
    nullkT = np.ascontiguousarray(null_kv[0].T).astype(BF16)
    nullv = np.ascontiguousarray(null_kv[1]).astype(BF16)
    lnw2 = np.ascontiguousarray(ln_w.reshape(8, 128).T)
    lnb2 = np.ascontiguousarray(ln_b.reshape(8, 128).T)

    in_maps = []
    for core in range(NCORES):
        g0 = HL * core
        sl = slice(g0 * DH, (g0 + HL) * DH)
        wqT = np.ascontiguousarray(Wq[sl, :].T).astype(BF16)
        woT = np.ascontiguousarray(Wo[:, sl].T).astype(BF16)
        biasT = np.zeros((HL, JPAD, N), dtype=BF16)
        for h in range(HL):
            biasT[h, 0:N, :] = attn_bias[g0 + h].T
        in_maps.append({
            "xT": xT, "wqT": wqT, "wkvT": wkvT, "woT": woT, "biasT": biasT,
            "lnw": lnw2, "lnb": lnb2, "nullkT": nullkT, "nullv": nullv,
        })
    return in_maps


def _install_ntff_hook():
    """Best-effort: register the axon NTFF profiling hook so trace=True works."""
    import types
    try:
        import antenv  # noqa: F401
        if "antenv.axon_hooks" not in sys.modules:
            mod = types.ModuleType("antenv.axon_hooks")
            mod._HOOK = None
            def _set(h):
                mod._HOOK = h
            def _get():
                return mod._HOOK
            mod.set_axon_ntff_profile_hook = _set
            mod.get_axon_ntff_profile_hook = _get
            sys.modules["antenv.axon_hooks"] = mod
        from trn_agent_boot.trn_boot import _ntff_profile_via_ctypes
        h = _ntff_profile_via_ctypes("/opt/axon/libaxon_pjrt.so")
        if h is not None:
            sys.modules["antenv.axon_hooks"].set_axon_ntff_profile_hook(h)
    except Exception:
        pass


def _run(inputs, trace=False):
    from concourse.bass_utils import run_bass_kernel_spmd

    if trace:
        _install_ntff_hook()
    nc = _get_nc()
    in_maps = _prep_in_maps(inputs)
    res = run_bass_kernel_spmd(
        nc, in_maps, core_ids=list(range(NCORES)), trace=trace
    )
    acc = res.results[0]["out"].astype(np.float64)
    for core in range(1, NCORES):
        acc += res.results[core]["out"]
    out = acc.astype(np.float32).reshape(B, N, DIM)
    return out, res


def kernel(**inputs):
    out, _ = _run(inputs, trace=False)
    return out
